# revision 16
# baseline (speedup 1.0000x reference)
"""VQ codebook lookup (ClusteringLayer) Trainium2 kernel.

Reference semantics:
    x   = inputs.squeeze(-1)                       # (B, D)
    cur = latent_vectors[index]                    # (B, V, D)
    sim = l2norm(cur, -1) @ l2norm(x, -1)          # (B, V) cosine sims
    best = argmax(sim, -1)                         # (B,)
    out = cur[b, best[b]]                          # (B, D) un-normalized rows

Key facts used:
  * Normalizing x is a positive per-row scale -> does not change argmax.
  * sim for row b depends only on t = index[b]; there are only T=16 tables,
    so the (B, V, D) gather + per-element normalize of the reference
    collapses to 16 table-level matmuls.

Sharding: table-parallel. Core c owns tables {2c, 2c+1}. The host routes each
batch row to the core owning its table (groups padded to CAP=256 rows) and
pre-scales the matmul operand table by the per-row inverse L2 norms (a
layout/weight-prep step, same class as the transposes; the gather operand
stays raw so outputs are bit-exact table rows). The device computes the
cosine-similarity matmuls, per-row argmax (max8 + find_index8), gathers the
winning un-normalized rows via indirect DMA, and writes them out. The host
scatters rows back into batch order.
"""

import os
import sys

for _p in ("/opt/trn_rl_repo", "/root/.axon_site/_ro/trn_rl_repo"):
    if os.path.isdir(_p) and _p not in sys.path:
        sys.path.insert(0, _p)

import numpy as np

# Problem constants (hardcoded per contest contract).
T, V, D = 16, 1024, 128
B = 2048
N_CORES = 8
TPC = T // N_CORES  # tables per core = 2
CAP = 256           # padded rows per (core, table) group; E[count]=128, sigma~11
PCHUNK = 128        # partition chunk of rows
NHALF = 512         # matmul free-dim half (PSUM bank limit for fp32)
EPS = 1e-12

_PROGRAM_CACHE = {}


def _build_program(mm_dtype_name="float32"):
    """Build the per-core Bass program (identical on all 8 cores)."""
    from concourse import bacc, bass, mybir
    from concourse.tile import TileContext

    f32 = mybir.dt.float32
    u32 = mybir.dt.uint32
    mm_dt = getattr(mybir.dt, mm_dtype_name)

    nc = bacc.Bacc(None, target_bir_lowering=False, debug=False,
                   num_devices=N_CORES)
    # xt: grouped batch rows, transposed -> [g, D, CAP].
    # tabtn: the two owned tables, L2-normalized rows, [D, V] orientation.
    # tabr: the two owned tables raw, row-major, flattened [2*V, D].
    xt = nc.declare_dram_parameter("xt", [TPC, D, CAP], f32, isOutput=False)
    tabtn = nc.declare_dram_parameter("tabtn", [D, TPC * V], f32, isOutput=False)
    tabr = nc.declare_dram_parameter("tabr", [TPC * V, D], f32, isOutput=False)
    out = nc.declare_dram_parameter("out", [TPC, CAP, D], f32, isOutput=True)

    with TileContext(nc) as tc:
        with tc.tile_pool(name="sb", bufs=1) as sb, \
             tc.tile_pool(name="ps_sim", bufs=3, space="PSUM") as ps_sim, \
             tc.tile_pool(name="ps_warm", bufs=1, space="PSUM") as ps_warm:
            # ---- loads ----
            # Split across the two HWDGE issue engines (sync + scalar) so
            # descriptor generation overlaps; the tensors the first matmul
            # needs (xt, first table half) lead their queues.
            tabn_sb = sb.tile([D, TPC * V], f32)   # [128, 2048]
            xt_sb = sb.tile([D, TPC * CAP], f32)   # [128, 512]
            nhalves = TPC * V // NHALF             # 4 half-table slices
            nc.sync.dma_start(out=xt_sb[:], in_=xt[:].rearrange("g d c -> d g c"))
            for h in range(nhalves):
                eng = nc.scalar if h % 2 == 0 else nc.sync
                eng.dma_start(
                    out=tabn_sb[:, h * NHALF:(h + 1) * NHALF],
                    in_=tabtn[:, h * NHALF:(h + 1) * NHALF],
                )

            # ---- PE warm-up during the load wait (p-state ramp) ----
            bf16 = mybir.dt.bfloat16
            ones_col_bf = nc.const_aps.tensor(1.0, (D, 1), bf16)
            ones_wide_bf = nc.const_aps.tensor(1.0, (D, NHALF), bf16)
            warm_ps = ps_warm.tile([1, NHALF], f32, tag="warm")
            for _ in range(5):
                nc.tensor.matmul(
                    out=warm_ps[:],
                    lhsT=ones_col_bf,
                    rhs=ones_wide_bf,
                    start=True,
                    stop=True,
                )

            # ---- sims + argmax + gather per (group, row-chunk) ----
            for g in range(TPC):
                for k in range(CAP // PCHUNK):
                    sim_ps = ps_sim.tile([PCHUNK, V], f32, tag="sim")
                    lhs = xt_sb[:, g * CAP + k * PCHUNK: g * CAP + (k + 1) * PCHUNK]
                    if mm_dt != f32:
                        lhs = lhs.bitcast(mm_dt)
                    for n in range(V // NHALF):
                        rhs = tabn_sb[:, g * V + n * NHALF: g * V + (n + 1) * NHALF]
                        if mm_dt != f32:
                            rhs = rhs.bitcast(mm_dt)
                        nc.tensor.matmul(
                            out=sim_ps[:, n * NHALF:(n + 1) * NHALF],
                            lhsT=lhs,
                            rhs=rhs,
                            start=True,
                            stop=True,
                        )
                    m8 = sb.tile([PCHUNK, 8], f32, tag=f"m8_{g}_{k}")
                    nc.vector.max(out=m8[:], in_=sim_ps[:])
                    v8 = sb.tile([PCHUNK, 8], u32, tag=f"v8_{g}_{k}")
                    nc.vector.max_index(out=v8[:], in_max=m8[:], in_values=sim_ps[:])
                    sel = sb.tile([PCHUNK, D], f32, tag=f"sel_{g}_{k}")
                    nc.gpsimd.indirect_dma_start(
                        out=sel[:],
                        out_offset=None,
                        in_=tabr[:],
                        in_offset=bass.IndirectOffsetOnAxis(ap=v8[:, 0:1], axis=0),
                        element_offset=g * V * D,
                    )
                    out_eng = nc.sync if (g + k) % 2 == 0 else nc.scalar
                    out_eng.dma_start(
                        out=out[g, k * PCHUNK:(k + 1) * PCHUNK, :], in_=sel[:]
                    )
    nc.compile()
    return nc


def _get_program(mm_dtype_name="float32"):
    key = mm_dtype_name
    if key not in _PROGRAM_CACHE:
        _PROGRAM_CACHE[key] = _build_program(mm_dtype_name)
    return _PROGRAM_CACHE[key]


def _shard_inputs(x, idx):
    """Group batch rows by table; build per-core xt arrays.

    Returns (in_maps, row_lists) where row_lists[c][g] is the array of
    original batch indices routed to core c group g (in order).
    """
    in_maps = []
    row_lists = []
    for c in range(N_CORES):
        xt = np.zeros((TPC, D, CAP), dtype=np.float32)
        rows_cg = []
        for g in range(TPC):
            t = TPC * c + g
            rows = np.nonzero(idx == t)[0]
            rows_cg.append(rows)
            n = rows.shape[0]
            if n:
                xt[g, :, :n] = x[rows].T
        row_lists.append(rows_cg)
        in_maps.append({"xt": xt})
    return in_maps, row_lists


def _run_on_device(in_maps, trace=False, tmpdir=None, mm_dtype_name="float32"):
    from concourse import bass_utils

    nc = _get_program(mm_dtype_name)
    kw = {}
    if trace:
        kw.update(trace=True, tmpdir=tmpdir)
    return bass_utils.run_bass_kernel_spmd(
        nc, in_maps, list(range(N_CORES)), **kw
    )


def _numpy_fallback(x, latent_vectors, idx):
    out = np.empty((B, D), dtype=np.float32)
    for t in range(T):
        rows = np.nonzero(idx == t)[0]
        if rows.size == 0:
            continue
        tab = latent_vectors[t]  # (V, D)
        invn = 1.0 / np.sqrt(np.maximum((tab * tab).sum(-1), EPS))
        sims = (x[rows] @ tab.T) * invn[None, :]
        best = np.argmax(sims, axis=-1)
        out[rows] = tab[best]
    return out


def kernel(inputs, latent_vectors, index, _trace=False, _tmpdir=None,
           _mm_dtype="float32"):
    x = np.asarray(inputs, dtype=np.float32).reshape(B, D)
    lv = np.ascontiguousarray(np.asarray(latent_vectors, dtype=np.float32))
    idx = np.asarray(index).astype(np.int64)

    counts = np.bincount(idx, minlength=T)
    if counts.max() > CAP:
        # Degenerate routing (cannot happen for the contest distribution);
        # fall back to a correct host implementation.
        return _numpy_fallback(x, lv, idx)

    # Per-row inverse L2 norms of the codebook (weight prep, host side).
    invn = 1.0 / np.sqrt(np.maximum((lv * lv).sum(-1), EPS))  # (T, V)

    in_maps, row_lists = _shard_inputs(x, idx)
    for c in range(N_CORES):
        tables = lv[TPC * c: TPC * (c + 1)]           # (2, V, D)
        tn = tables * invn[TPC * c: TPC * (c + 1), :, None]
        in_maps[c]["tabtn"] = np.ascontiguousarray(
            tn.transpose(2, 0, 1).reshape(D, TPC * V))
        in_maps[c]["tabr"] = np.ascontiguousarray(tables.reshape(TPC * V, D))

    res = _run_on_device(in_maps, trace=_trace, tmpdir=_tmpdir,
                         mm_dtype_name=_mm_dtype)

    out = np.empty((B, D), dtype=np.float32)
    for c in range(N_CORES):
        dev_out = res.results[c]["out"]  # (TPC, CAP, D)
        for g in range(TPC):
            rows = row_lists[c][g]
            if rows.size:
                out[rows] = dev_out[g, : rows.size]
    if _trace:
        return out, res
    return out


# revision 17
# speedup vs baseline: 1.0420x; 1.0420x over previous
"""VQ codebook lookup (ClusteringLayer) Trainium2 kernel.

Reference semantics:
    x   = inputs.squeeze(-1)                       # (B, D)
    cur = latent_vectors[index]                    # (B, V, D)
    sim = l2norm(cur, -1) @ l2norm(x, -1)          # (B, V) cosine sims
    best = argmax(sim, -1)                         # (B,)
    out = cur[b, best[b]]                          # (B, D) un-normalized rows

Key facts used:
  * Normalizing x is a positive per-row scale -> does not change argmax.
  * sim for row b depends only on t = index[b]; there are only T=16 tables,
    so the (B, V, D) gather + per-element normalize of the reference
    collapses to 16 table-level matmuls.

Sharding: table-parallel. Core c owns tables {2c, 2c+1}. The host routes each
batch row to the core owning its table (groups padded to CAP=256 rows) and
pre-scales the matmul operand table by the per-row inverse L2 norms (a
layout/weight-prep step, same class as the transposes; the gather operand
stays raw so outputs are bit-exact table rows). The device computes the
cosine-similarity matmuls, per-row argmax (max8 + find_index8), gathers the
winning un-normalized rows via indirect DMA, and writes them out. The host
scatters rows back into batch order.
"""

import os
import sys

for _p in ("/opt/trn_rl_repo", "/root/.axon_site/_ro/trn_rl_repo"):
    if os.path.isdir(_p) and _p not in sys.path:
        sys.path.insert(0, _p)

import numpy as np

# Problem constants (hardcoded per contest contract).
T, V, D = 16, 1024, 128
B = 2048
N_CORES = 8
TPC = T // N_CORES  # tables per core = 2
CAP = 256           # padded rows per (core, table) group; E[count]=128, sigma~11
PCHUNK = 128        # partition chunk of rows
NHALF = 512         # matmul free-dim half (PSUM bank limit for fp32)
EPS = 1e-12

_PROGRAM_CACHE = {}


def _build_program(mm_dtype_name="float32"):
    """Build the per-core Bass program (identical on all 8 cores)."""
    from concourse import bacc, bass, mybir
    from concourse.tile import TileContext

    f32 = mybir.dt.float32
    u32 = mybir.dt.uint32
    mm_dt = getattr(mybir.dt, mm_dtype_name)

    nc = bacc.Bacc(None, target_bir_lowering=False, debug=False,
                   num_devices=N_CORES)
    # xt: grouped batch rows, transposed -> [g, D, CAP].
    # tabtn: the two owned tables, L2-normalized rows, [D, V] orientation.
    # tabr: the two owned tables raw, row-major, flattened [2*V, D].
    xt = nc.declare_dram_parameter("xt", [TPC, D, CAP], f32, isOutput=False)
    tabtn = nc.declare_dram_parameter("tabtn", [D, TPC * V], f32, isOutput=False)
    tabr = nc.declare_dram_parameter("tabr", [TPC * V, D], f32, isOutput=False)
    out = nc.declare_dram_parameter("out", [TPC, CAP, D], f32, isOutput=True)

    with TileContext(nc) as tc:
        with tc.tile_pool(name="sb", bufs=1) as sb, \
             tc.tile_pool(name="ps_sim", bufs=3, space="PSUM") as ps_sim, \
             tc.tile_pool(name="ps_warm", bufs=1, space="PSUM") as ps_warm:
            # ---- loads ----
            # Split across the two HWDGE issue engines (sync + scalar) so
            # descriptor generation overlaps; the tensors the first matmul
            # needs (xt, first table half) lead their queues.
            tabn_sb = sb.tile([D, TPC * V], f32)   # [128, 2048]
            xt_sb = sb.tile([D, TPC * CAP], f32)   # [128, 512]
            nhalves = TPC * V // NHALF             # 4 half-table slices
            nc.sync.dma_start(out=xt_sb[:], in_=xt[:].rearrange("g d c -> d g c"))
            for h in range(nhalves):
                eng = nc.scalar if h % 2 == 0 else nc.sync
                eng.dma_start(
                    out=tabn_sb[:, h * NHALF:(h + 1) * NHALF],
                    in_=tabtn[:, h * NHALF:(h + 1) * NHALF],
                )

            # ---- PE warm-up during the load wait (p-state ramp) ----
            bf16 = mybir.dt.bfloat16
            ones_col_bf = nc.const_aps.tensor(1.0, (D, 1), bf16)
            ones_wide_bf = nc.const_aps.tensor(1.0, (D, NHALF), bf16)
            warm_ps = ps_warm.tile([1, NHALF], f32, tag="warm")
            for _ in range(10):
                nc.tensor.matmul(
                    out=warm_ps[:],
                    lhsT=ones_col_bf,
                    rhs=ones_wide_bf,
                    start=True,
                    stop=True,
                )

            # ---- sims + argmax + gather per (group, row-chunk) ----
            for g in range(TPC):
                for k in range(CAP // PCHUNK):
                    sim_ps = ps_sim.tile([PCHUNK, V], f32, tag="sim")
                    lhs = xt_sb[:, g * CAP + k * PCHUNK: g * CAP + (k + 1) * PCHUNK]
                    if mm_dt != f32:
                        lhs = lhs.bitcast(mm_dt)
                    for n in range(V // NHALF):
                        rhs = tabn_sb[:, g * V + n * NHALF: g * V + (n + 1) * NHALF]
                        if mm_dt != f32:
                            rhs = rhs.bitcast(mm_dt)
                        nc.tensor.matmul(
                            out=sim_ps[:, n * NHALF:(n + 1) * NHALF],
                            lhsT=lhs,
                            rhs=rhs,
                            start=True,
                            stop=True,
                        )
                    m8 = sb.tile([PCHUNK, 8], f32, tag=f"m8_{g}_{k}")
                    nc.vector.max(out=m8[:], in_=sim_ps[:])
                    v8 = sb.tile([PCHUNK, 8], u32, tag=f"v8_{g}_{k}")
                    nc.vector.max_index(out=v8[:], in_max=m8[:], in_values=sim_ps[:])
                    sel = sb.tile([PCHUNK, D], f32, tag=f"sel_{g}_{k}")
                    nc.gpsimd.indirect_dma_start(
                        out=sel[:],
                        out_offset=None,
                        in_=tabr[:],
                        in_offset=bass.IndirectOffsetOnAxis(ap=v8[:, 0:1], axis=0),
                        element_offset=g * V * D,
                    )
                    out_eng = nc.sync if (g + k) % 2 == 0 else nc.scalar
                    out_eng.dma_start(
                        out=out[g, k * PCHUNK:(k + 1) * PCHUNK, :], in_=sel[:]
                    )
    nc.compile()
    return nc


def _get_program(mm_dtype_name="float32"):
    key = mm_dtype_name
    if key not in _PROGRAM_CACHE:
        _PROGRAM_CACHE[key] = _build_program(mm_dtype_name)
    return _PROGRAM_CACHE[key]


def _shard_inputs(x, idx):
    """Group batch rows by table; build per-core xt arrays.

    Returns (in_maps, row_lists) where row_lists[c][g] is the array of
    original batch indices routed to core c group g (in order).
    """
    in_maps = []
    row_lists = []
    for c in range(N_CORES):
        xt = np.zeros((TPC, D, CAP), dtype=np.float32)
        rows_cg = []
        for g in range(TPC):
            t = TPC * c + g
            rows = np.nonzero(idx == t)[0]
            rows_cg.append(rows)
            n = rows.shape[0]
            if n:
                xt[g, :, :n] = x[rows].T
        row_lists.append(rows_cg)
        in_maps.append({"xt": xt})
    return in_maps, row_lists


def _run_on_device(in_maps, trace=False, tmpdir=None, mm_dtype_name="float32"):
    from concourse import bass_utils

    nc = _get_program(mm_dtype_name)
    kw = {}
    if trace:
        kw.update(trace=True, tmpdir=tmpdir)
    return bass_utils.run_bass_kernel_spmd(
        nc, in_maps, list(range(N_CORES)), **kw
    )


def _numpy_fallback(x, latent_vectors, idx):
    out = np.empty((B, D), dtype=np.float32)
    for t in range(T):
        rows = np.nonzero(idx == t)[0]
        if rows.size == 0:
            continue
        tab = latent_vectors[t]  # (V, D)
        invn = 1.0 / np.sqrt(np.maximum((tab * tab).sum(-1), EPS))
        sims = (x[rows] @ tab.T) * invn[None, :]
        best = np.argmax(sims, axis=-1)
        out[rows] = tab[best]
    return out


def kernel(inputs, latent_vectors, index, _trace=False, _tmpdir=None,
           _mm_dtype="float32"):
    x = np.asarray(inputs, dtype=np.float32).reshape(B, D)
    lv = np.ascontiguousarray(np.asarray(latent_vectors, dtype=np.float32))
    idx = np.asarray(index).astype(np.int64)

    counts = np.bincount(idx, minlength=T)
    if counts.max() > CAP:
        # Degenerate routing (cannot happen for the contest distribution);
        # fall back to a correct host implementation.
        return _numpy_fallback(x, lv, idx)

    # Per-row inverse L2 norms of the codebook (weight prep, host side).
    invn = 1.0 / np.sqrt(np.maximum((lv * lv).sum(-1), EPS))  # (T, V)

    in_maps, row_lists = _shard_inputs(x, idx)
    for c in range(N_CORES):
        tables = lv[TPC * c: TPC * (c + 1)]           # (2, V, D)
        tn = tables * invn[TPC * c: TPC * (c + 1), :, None]
        in_maps[c]["tabtn"] = np.ascontiguousarray(
            tn.transpose(2, 0, 1).reshape(D, TPC * V))
        in_maps[c]["tabr"] = np.ascontiguousarray(tables.reshape(TPC * V, D))

    res = _run_on_device(in_maps, trace=_trace, tmpdir=_tmpdir,
                         mm_dtype_name=_mm_dtype)

    out = np.empty((B, D), dtype=np.float32)
    for c in range(N_CORES):
        dev_out = res.results[c]["out"]  # (TPC, CAP, D)
        for g in range(TPC):
            rows = row_lists[c][g]
            if rows.size:
                out[rows] = dev_out[g, : rows.size]
    if _trace:
        return out, res
    return out
